# revision 62
# baseline (speedup 1.0000x reference)
"""Trainium2 Bass kernel for nn_Ani_layer (dense_cnn), v6 (~64.5us,
vs 65.7us baseline; measured range 64.5-65.3 across runs).

A 64->64ch 3x3 conv whose weight is built from params x basis, with
per-window mean subtraction folded into the conv weights, a vector-norm
"relu" epilogue (out/norm masked where norm<=b) and mean re-add.

Distribution: 8 shards = (batch b in 0..3) x (H half in 0..1); each core
gets a pre-padded bf16 (64ch, 67, 130) input slab and produces
(64ch, 64, 128) bf16 (host upconverts to fp32). No collectives.

Structure:
  - Input: 3 chunks per stream split over sync+scalar HWDGE queues plus
    the gpsimd swdge for the late chunks (WAW-gated behind chunk 1 via
    an overlapping row so they don't steal early bus bandwidth). ACT
    tables warmed by dummy activations mid-load (Rsqrt first: the
    reciprocal_sqrt set also covers Square/Copy).
  - PSUM: 2-bank tiles [128, 2, 512] (one batch = 2 groups of 3 output
    rows, N=390 contiguous windows per bank), ring depth 4.
  - Conv tap-major over 2-batch pairs (20-mm bursts; LDWEIGHTS shared):
    a post-legalize pass drops InstLdweights whose weights AP repeats
    the previous load (121 -> 60; InstMatmult.ldweights=False is only a
    split marker, so dedup must edit the instruction stream).
  - Modulo schedule at pair granularity: iteration I emits stage2(pair
    I), stage1(pair I+1), conv(pair I+2); no engine's program order
    serializes the epilogue chain, and PSUM slot reuse has a full
    iteration of slack. The odd (3,1)-row batch sits mid-schedule.
  - stage1: ACT Square (all 64 conv partitions) -> DVE copy of v1
    squares to base partition 0 (walrus requires equal SB base
    partitions for TensorTensor) -> DVE TT add -> ACT Rsqrt (raw LUT)
    -> DVE dup -> DVE custom STT select(r < 1/b, (conv+bias)*r, 0)
    (the norm<=b mask via monotonicity of rsqrt; r=inf/nan from garbage
    windows lands in the masked arm).
  - stage2: all batches path B = one DVE TT add (PSUM avg rows + m)
    writing bf16 out. The PE stream is pure conv matmuls: per-batch
    id-matmul stalls flip the HAM clock gate (K=4/8, 1.2 vs 2.4 GHz)
    every activity window in some device states (measured 66.7 -> 80.8
    on identical code); all-B holds K=8 for 13-17us stretches. Offload
    attempts (sync-DMA r-dup, gpsimd sqb copy, A/B alternation) all
    measured slower despite better paper engine balance.
  - Output bf16, one DMA descriptor per batch, host upconverts.
"""

import os
import sys
from contextlib import ExitStack

for _p in ("/opt/trn_rl_repo", os.path.expanduser("~/.axon_site/_ro/trn_rl_repo")):
    if os.path.isdir(_p) and _p not in sys.path:
        sys.path.insert(0, _p)

import numpy as np
import ml_dtypes

import concourse.bass as bass
import concourse.bacc as bacc
import concourse.tile as tile
import concourse.dve_ops as dve_ops_mod
from concourse import mybir
from concourse.bass_utils import run_bass_kernel_spmd
from concourse.dve_spec import C0, C1, C2, Spec, Src0, Src1, Zero, lower, select
from concourse.dve_spec import _has_src1
from concourse.dve_uop import DveOpSpec

F32 = mybir.dt.float32
BF16 = mybir.dt.bfloat16
ALU = mybir.AluOpType
ACTF = mybir.ActivationFunctionType

B, O, I, KS, H, W = 4, 32, 32, 3, 128, 128
NCH = 2 * I          # 64 input channels
HS = H // 2          # 64 output rows per shard
PH, PW = HS + 2, W + 2   # padded shard: 66 x 130
N_CORES = 8

DUP_ON_SYNC = False  # r64 dup via sync SBUF->SBUF DMA (else DVE copy)
LDW_DEDUP = True     # drop repeated-weights InstLdweights post-legalize

# batches of 2 PSUM banks; 10 of (3,3) rows + final (3,1).
BATCHES = [[(6 * p, 3), (6 * p + 3, 3)] for p in range(10)] + [[(60, 3), (63, 1)]]
NB = len(BATCHES)


def _register_dve_op(name, spec):
    for op in dve_ops_mod.OPS:
        if op.name == name:
            return op
    row = dve_ops_mod._CUSTOM_DVE_ROW_BASE + len(dve_ops_mod.OPS)
    assert row < 0x20
    dve_ops_mod._SUB_OPCODE_FOR_NAME[name] = row
    uops = lower(spec, ver="v3")
    sha = DveOpSpec(name=name, opcode=row, uops=uops,
                    rd1_en=_has_src1(spec)).sha("v3")
    op = dve_ops_mod.DveOp(name, spec, subdim=False, uops_sha={"v3": sha})
    dve_ops_mod.OPS.append(op)
    dve_ops_mod.CUSTOM_DVE_SPECS[name] = spec
    return op


def _stt_mask_op():
    # m = (conv + bias) * r where r < 1/b (norm > b), else 0. The norm<=b
    # mask of the reference is equivalent to r = rsqrt(n2) >= 1/b by
    # monotonicity; r=inf/nan (n2=0 garbage) also lands in the masked arm.
    body = select(Src1 < C1, (Src0 + C0) * Src1, Zero)

    def ref(in0, in1, c0, c1, c2):
        r = in1.astype(np.float32)
        m = (in0.astype(np.float32) + c0) * r
        return np.where(r < c1, m, 0.0)

    return _register_dve_op("STT_MASK_ANT", Spec(body=body, reference=ref))


def _act_raw(nc, out, in_, func, bias_ap, scale):
    """Emit InstActivation directly (bass bans Rsqrt; the reciprocal_sqrt
    LUT measured ~5e-5 max rel err over [1e-4, 1e2])."""
    eng = nc.scalar
    inputs = [eng.lower_ap(in_), eng.lower_ap(bias_ap),
              mybir.ImmediateValue(dtype=mybir.dt.float32, value=scale),
              mybir.ImmediateValue(dtype=mybir.dt.float32, value=0.0)]
    return eng.add_instruction(mybir.InstActivation(
        name=nc.get_next_instruction_name(), func=func,
        ins=inputs, outs=[eng.lower_ap(out)]))


def _dedup_ldweights(nc):
    """Remove InstLdweights whose weights AP repeats the previous PE
    weight load (the PE array still holds them); the paired matmult
    inherits the dropped load's dependency edges."""
    removed = 0
    for f in nc.m.functions:
        for blk in f.blocks:
            insts = list(blk.instructions)
            if not any(isinstance(i, mybir.InstLdweights) for i in insts):
                continue
            out, remap = [], {}
            prev_key, pending = None, None
            for i in insts:
                if isinstance(i, mybir.InstLdweights):
                    key = (str(i.ins[0]), str(i.tile_position),
                           str(i.tile_size), str(i.perf_mode))
                    if key == prev_key:
                        pending = i
                        removed += 1
                        continue
                    prev_key = key
                elif isinstance(i, mybir.InstMatmult):
                    if pending is not None:
                        i.merge_dependencies_from(pending)
                        remap[pending.name] = i.name
                        pending = None
                out.append(i)
            assert pending is None
            if remap:
                for i in out:
                    i.remap_dependency_names(remap)
                blk.instructions = out
    return removed


_NC = {}


def _build_nc(rinv):
    op_mask = _stt_mask_op()

    nc = bacc.Bacc("TRN2")
    # x slab has PH+1 rows so the merged lo/hi descriptor (hi = rows r+1)
    # never reads out of bounds; row PH is garbage, never consumed.
    x_d = nc.declare_dram_parameter("x", [NCH, PH + 1, PW], BF16,
                                    isOutput=False)
    wp_d = nc.declare_dram_parameter("wp", [3, 128, 128], BF16, isOutput=False)
    wrb_d = nc.declare_dram_parameter("wrb", [128, 128], BF16, isOutput=False)
    wr2_d = nc.declare_dram_parameter("wr2", [NCH, 128], BF16, isOutput=False)
    id_d = nc.declare_dram_parameter("idm", [NCH, NCH], BF16, isOutput=False)
    cst_d = nc.declare_dram_parameter("cst", [NCH, 1], F32, isOutput=False)
    out_d = nc.declare_dram_parameter("out", [NCH, HS * W], BF16, isOutput=True)

    with tile.TileContext(nc) as tc, ExitStack() as ctx:
        singles = ctx.enter_context(tc.tile_pool(name="singles", bufs=1))
        psum = ctx.enter_context(tc.tile_pool(name="psum", bufs=4, space="PSUM"))
        ep = ctx.enter_context(tc.tile_pool(name="ep", bufs=4))
        outp = ctx.enter_context(tc.tile_pool(name="outp", bufs=4))

        xt = singles.tile([128, PH + 1, PW], BF16, tag="xt")
        xb = singles.tile([128, PH, PW], BF16, tag="xb")
        wp_s = singles.tile([128, 3, 128], BF16, tag="wp")
        wrb_s = singles.tile([128, 128], BF16, tag="wrb")
        wr2_s = singles.tile([NCH, 128], BF16, tag="wr2")
        id_s = singles.tile([NCH, NCH], BF16, tag="idm")
        cst = singles.tile([NCH, 1], F32, tag="cst")
        zb = singles.tile([O, 1], F32, tag="zb")
        nc.vector.memset(zb, 0.0)
        # group (63,1)'s single-tap matmul reads 2 elements into xt row 66
        nc.vector.memset(xt[0:NCH, PH:PH + 1, :], 0.0)

        # --- input loads: one merged lo+hi descriptor per chunk ---------
        # xt = [x ; x shifted down one row]: src partition dim is the
        # composite (row-shift h in 0..1, channel c in 0..63) built as a
        # raw overlapping-stride AP. xb = [x ; x shifted left one column]
        # likewise with a column-shift partition dim (129 columns).
        # chunk 2 overlaps chunk 1's last row: the WAW dep serializes the
        # gpsimd late-chunk DMAs behind chunk 1, keeping early bus
        # bandwidth for the critical first chunks. Chunk 0 covers all of
        # conv pair [0,1] (rows <= 13) so the first burst never stalls.
        CH = [(0, 14), (14, 38), (37, PH)]

        def xt_lo(q, r0, r1):
            q.dma_start(out=xt[0:NCH, r0:r1, :], in_=x_d[:, r0:r1, :])

        def xt_hi(q, r0, r1):
            q.dma_start(out=xt[NCH:128, r0:r1, :], in_=x_d[:, r0 + 1:r1 + 1, :])

        def xb_lo(q, r0, r1):
            r0 = max(r0, 2)
            q.dma_start(out=xb[0:NCH, r0:r1, :], in_=x_d[:, r0:r1, :])

        def xb_hi(q, r0, r1):
            r0 = max(r0, 2)
            q.dma_start(out=xb[NCH:128, r0:r1, 0:PW - 1],
                        in_=x_d[:, r0:r1, 1:PW])

        nc.vector.memset(xb[NCH:128, :, PW - 1:PW], 0.0)
        nc.sync.dma_start(out=wp_s, in_=wp_d.rearrange("j k m -> k j m"))
        xt_lo(nc.sync, *CH[0])
        xt_hi(nc.scalar, *CH[0])
        nc.scalar.dma_start(out=wrb_s, in_=wrb_d[:, :])
        xb_lo(nc.sync, *CH[0])
        nc.scalar.dma_start(out=wr2_s, in_=wr2_d[:, :])
        xb_hi(nc.scalar, *CH[0])
        nc.sync.dma_start(out=cst, in_=cst_d[:, :])

        # warm the ACT tables during the input-load window. Rsqrt FIRST so
        # the pass picks the reciprocal_sqrt set, which also contains
        # square and copy. zs is scratch: Rsqrt(0)=inf must not land in
        # zb, which real Rsqrts use as bias.
        zs = singles.tile([O, 1], F32, tag="zs")
        _act_raw(nc, zs, zb, ACTF.Rsqrt, zb, 1.0)
        nc.scalar.activation(zs, zb, ACTF.Square, bias=zb, scale=1.0)
        nc.scalar.activation(zs, zb, ACTF.Copy)

        xt_lo(nc.sync, *CH[1])
        xt_hi(nc.scalar, *CH[1])
        xb_lo(nc.sync, *CH[1])
        xb_hi(nc.scalar, *CH[1])
        for r0, r1 in CH[2:]:
            xt_lo(nc.gpsimd, r0, r1)
            xt_hi(nc.gpsimd, r0, r1)
            xb_lo(nc.gpsimd, r0, r1)
            xb_hi(nc.gpsimd, r0, r1)
        nc.gpsimd.dma_start(out=id_s, in_=id_d[:, :])

        xtf = xt.rearrange("p a b -> p (a b)")
        xbf = xb.rearrange("p a b -> p (a b)")

        def conv_group(bp):
            """Tap-major conv over a group of batches (2-batch bursts keep
            the PE dense past the HAM activity window); repeated weight
            loads are stripped by _dedup_ldweights after legalization."""
            tiles = {p: psum.tile([128, 2, 512], F32, tag="pt", name="pt")
                     for p in bp}
            for t in range(5):
                for p in bp:
                    pt = tiles[p]
                    for bi, (h0, nr) in enumerate(BATCHES[p]):
                        N = nr * PW
                        if t < 3:
                            nc.tensor.matmul(
                                pt[:, bi, 0:N], wp_s[:, t, :],
                                xtf[:, h0 * PW + t:h0 * PW + t + N],
                                start=(t == 0), stop=False)
                        elif t == 3:
                            nc.tensor.matmul(
                                pt[:, bi, 0:N], wrb_s,
                                xbf[:, (h0 + 2) * PW:(h0 + 2) * PW + N],
                                start=False, stop=False)
                        else:
                            nc.tensor.matmul(
                                pt[:, bi, 0:N], wr2_s,
                                xtf[0:NCH,
                                    (h0 + 2) * PW + 2:(h0 + 2) * PW + 2 + N],
                                start=False, stop=True)
            return tiles

        def subunits(p):
            banks = BATCHES[p]
            if banks[0][1] == banks[1][1]:
                return [(0, 2, banks[0][1] * PW)]
            return [(0, 1, banks[0][1] * PW), (1, 1, banks[1][1] * PW)]

        ms = {}

        def stage1(p, pt):
            mh = ep.tile([NCH, 2, 390], BF16, tag="m")
            for b0, nb, N in subunits(p):
                pc = pt[0:NCH, b0:b0 + nb, 0:N]
                sqh = ep.tile([NCH, nb, N], BF16, tag="sq")
                nc.scalar.activation(sqh, pc, ACTF.Square, bias=cst,
                                     scale=1.0)
                sqb = ep.tile([O, nb, N], BF16, tag="sqb")
                nc.vector.tensor_copy(sqb, sqh[O:NCH])
                n2h = ep.tile([O, nb, N], BF16, tag="n2")
                nc.vector.tensor_tensor(out=n2h, in0=sqh[0:O], in1=sqb,
                                        op=ALU.add)
                rh = ep.tile([NCH, nb, N], BF16, tag="r")
                _act_raw(nc, rh[0:O], n2h, ACTF.Rsqrt, zb, 1.0)
                if DUP_ON_SYNC and nb == 2 and p != 9:
                    # tail batches dup on DVE: the sync round trip adds
                    # ~2.5us of chain latency with nothing left to hide it
                    nc.sync.dma_start(out=rh[O:NCH], in_=rh[0:O])
                else:
                    nc.vector.tensor_copy(rh[O:NCH], rh[0:O])
                nc.vector._custom_dve(op_mask, out=mh[:, b0:b0 + nb, 0:N],
                                      in0=pc, in1=rh, s0=cst, s1=rinv)
            ms[p] = mh

        def stage2(p, pt):
            mh = ms.pop(p)
            ot = outp.tile([NCH, 2, 390], BF16, tag="ot")
            for si, (b0, nb, N) in enumerate(subunits(p)):
                pa = pt[NCH:128, b0:b0 + nb, 0:N]
                # A on even batches, B otherwise (the last batch's nb=1
                # subunits fall through to B).
                path_a = False
                if path_a:
                    for bi in range(nb):
                        bk = b0 + bi
                        Nn = BATCHES[p][bk][1] * PW
                        nc.tensor.matmul(
                            pt[NCH:128, bk, 0:Nn], id_s, mh[:, bk, 0:Nn],
                            start=False, stop=True, tile_position=(0, 64))
                    nc.scalar.activation(ot[:, b0:b0 + nb, 0:N], pa,
                                         ACTF.Copy)
                else:
                    nc.vector.tensor_tensor(out=ot[:, b0:b0 + nb, 0:N],
                                            in0=pa,
                                            in1=mh[:, b0:b0 + nb, 0:N],
                                            op=ALU.add)
            otv = ot.rearrange("p b (r c) -> p b r c", c=PW)
            h0 = BATCHES[p][0][0]
            if BATCHES[p][1][1] == 3:
                nc.sync.dma_start(out=out_d[:, h0 * W:(h0 + 6) * W],
                                  in_=otv[:, :, 0:3, 0:W])
            else:
                nc.sync.dma_start(out=out_d[:, h0 * W:(h0 + 3) * W],
                                  in_=otv[:, 0, 0:3, 0:W])
                nc.sync.dma_start(out=out_d[:, (h0 + 3) * W:(h0 + 4) * W],
                                  in_=otv[:, 1, 0:1, 0:W])

        # modulo schedule at pair granularity: iteration I emits
        # stage2(pair I) (PE id-mms first, freeing this pair's PSUM
        # slots), stage1(pair I+1), then the conv burst for pair I+2.
        # Batch 10 (the per-bank half pair with the slow two-subunit
        # chain) sits mid-schedule so the drain tail is one clean batch.
        PAIRS = [[0, 1], [2, 3], [4, 10], [5, 6], [7, 8], [9]]
        pts = {}
        pts.update(conv_group(PAIRS[0]))
        pts.update(conv_group(PAIRS[1]))
        for p in PAIRS[0]:
            stage1(p, pts[p])
        for i, pair in enumerate(PAIRS):
            for p in pair:
                stage2(p, pts[p])
            if i + 1 < len(PAIRS):
                for p in PAIRS[i + 1]:
                    stage1(p, pts[p])
            if i + 2 < len(PAIRS):
                pts.update(conv_group(PAIRS[i + 2]))

    if LDW_DEDUP:
        _dedup_ldweights(nc)
    nc.compile()
    return nc


def _get_nc(rinv):
    key = float(rinv)
    if key not in _NC:
        _NC[key] = _build_nc(key)
    return _NC[key]


def _prep(params, basis, bias_term, b):
    params = np.asarray(params, np.float32)
    basis = np.asarray(basis, np.float32)
    Kr = np.einsum("abcd,cdefgh->abefgh", params, basis)  # (O,I,K,K,2,2)
    kern = Kr.transpose(0, 4, 1, 5, 2, 3).reshape(2 * O, 2 * I, KS, KS)
    # reference pairs patch (kh=q, kw=p) with kern[o2, c, p, q]:
    Wtap = kern.transpose(0, 1, 3, 2)  # [o2, c, dh, dw]
    # fold per-window mean subtraction into the weights
    Ksum = np.stack([Wtap[:, 0::2].sum(axis=(1, 2, 3)),
                     Wtap[:, 1::2].sum(axis=(1, 2, 3))], axis=1)  # [o2, 2]
    cpar = np.arange(NCH) % 2
    Wp = Wtap - (Ksum[:, cpar] / float(I * KS * KS))[:, :, None, None]
    # device output order: dev channel = 32*v + o  <->  torch channel 2*o + v
    perm = np.array([2 * (i % O) + i // O for i in range(NCH)])
    Wdev = np.zeros((128, NCH, KS, KS), np.float32)
    Wdev[0:NCH] = Wp[perm]
    avg_w = np.zeros((NCH, NCH, KS, KS), np.float32)
    for v in (0, 1):
        avg_w[O * v:O * v + O, v::2, :, :] = 1.0 / float(I * KS * KS)
    Wdev[NCH:128] = avg_w
    wp = np.zeros((3, 128, 128), np.float32)
    wr = np.zeros((3, NCH, 128), np.float32)
    for j in range(3):
        wp[j, 0:NCH, :] = Wdev[:, :, 0, j].T
        wp[j, NCH:128, :] = Wdev[:, :, 1, j].T
        wr[j, :, :] = Wdev[:, :, 2, j].T
    wrb = np.concatenate([wr[0], wr[1]], axis=0)  # [128, 128]
    bt = np.asarray(bias_term, np.float32).reshape(O, 2)
    cst = np.zeros((NCH, 1), np.float32)
    for v in (0, 1):
        cst[O * v:O * v + O, 0] = bt[:, v]
    rinv = 1.0 / float(np.asarray(b).reshape(-1)[0])
    return (wp.astype(ml_dtypes.bfloat16), wrb.astype(ml_dtypes.bfloat16),
            wr[2].astype(ml_dtypes.bfloat16), cst, rinv, perm)


def _run(inputs, trace=False):
    xx = np.asarray(inputs["xx"], np.float32)
    wp, wrb, wr2, cst, rinv, perm = _prep(inputs["params"], inputs["basis"],
                                          inputs["bias_term"], inputs["b"])
    # one extra bottom row so the merged lo/hi input descriptor (hi reads
    # rows r+1) stays in bounds for the last chunk
    xp = np.pad(xx, ((0, 0), (0, 0), (1, 2), (1, 1)), mode="edge")
    xpb = xp.astype(ml_dtypes.bfloat16)
    idm = np.eye(NCH, dtype=ml_dtypes.bfloat16)
    in_maps = []
    for core in range(N_CORES):
        bb, half = core // 2, core % 2
        shard = np.ascontiguousarray(
            xpb[bb, :, half * HS:half * HS + PH + 1, :])
        in_maps.append({"x": shard, "wp": wp, "wrb": wrb, "wr2": wr2,
                        "idm": idm, "cst": cst})
    nc = _get_nc(rinv)
    res = run_bass_kernel_spmd(nc, in_maps, list(range(N_CORES)), trace=trace)
    out = np.zeros((B, NCH, H, W), np.float32)
    for core in range(N_CORES):
        bb, half = core // 2, core % 2
        dev = np.asarray(res.results[core]["out"]).astype(np.float32)
        dev = dev.reshape(NCH, HS, W)
        out[bb, perm, half * HS:(half + 1) * HS, :] = dev
    return out, res.exec_time_ns


def kernel(**inputs):
    out, _ = _run(inputs, trace=False)
    return out
